# revision 10
# baseline (speedup 1.0000x reference)
"""Trainium2 Bass kernel for nn_BitModel (MLGRU step + BitGLU, ternary weights).

Strategy: pure data-parallel over the 4096 batch dim (512 rows per core,
zero collectives). Weights are ternarized exactly in f32 on the host;
ternary values {-1, 0, +1} are exact in fp16 so device matmuls run at
full 1-cycle/row TensorE rate.

Key restructure vs the fp16 reference pipeline: out_proj is FOLDED into
proj_u / proj_g on the host (M_u = tern(W_u) @ tern(W_o), integer
entries |M| <= ~165, exact in fp16; effective biases folded likewise).
This removes the out_proj matmul stream AND removes one of the fp16
quantization points feeding the proj_g sigmoid, whose saturation-
boundary flips dominate the error budget (t_g2 has std ~1.8e4; a
relative eps on the input becomes ~0.7*sqrt(eps) output error).

Precision scheme (emulated rel err ~1.06e-2, budget 2e-2):
- x feeding f/g gates: fp16 hi + e4m3 lo (x-side flips are the worst).
- x feeding c: fp16 only.
- gh (feeds folded M_u/M_g): fp16.
- gu (feeds proj_out): fp16, pre-scaled by 1/16 for range.
All accumulation f32 in PSUM. Per-core matmul instruction count 4096
(768 fp16 + 256 fp8-DR in phase 1, 2048 + 1024 fp16 in phases 2/3).
"""

import sys

sys.path.insert(0, "/opt/trn_rl_repo")

import numpy as np

import concourse.bass as bass
import concourse.mybir as mybir
import concourse.tile as tile
from concourse.vector_clock import ScopedClock

DIM = 2048
HID = 8192
BATCH = 4096
NCORES = 8
B = BATCH // NCORES  # 512 batch rows per core
P = 128
JC_D = DIM // P  # 16 feature chunks for DIM
JC_H = HID // P  # 64 feature chunks for HID
THRESH = 0.33
GU_SCALE = 16.0  # gu is stored as gu/16 in fp16 to stay inside fp16 range

F16 = mybir.dt.float16
F32 = mybir.dt.float32
F8 = mybir.dt.float8e4  # ml_dtypes.float8_e4m3 (values kept < 240: TRN max)
XLO_SCALE = 512.0  # x_lo is shipped as e4m3(512*x_lo); the fp8 gate weights
W8_SCALE = 2.0 ** -9  # carry the inverse scale (+-2^-9, exact e4m3 subnormals)

# bias column layout in the packed [128, 192] bias tensor
COL_NF = 0  # -f_gate_b (negated: we compute 1-f = sigmoid(-(t+b)))
COL_C = 16
COL_G = 32
COL_U = 48  # effective: tern(W_u) @ out_proj_b + proj_u_b
COL_G2 = 112  # effective: tern(W_g2) @ out_proj_b + proj_g_b
COL_Y = 176
N_BIAS_COLS = 192


def _patch_tile_drain():
    """This walrus build rejects instructions carrying >~2 attached sem
    waits ("Too many sync wait commands") and Tile's kernel-tail drain
    carries one wait per active logical proc. Re-emit those waits as
    standalone wait_ge instructions (1 wait each) before a wait-free
    drain."""
    if getattr(tile.TileContext, "_drain_patched", False):
        return

    def _drain_and_barrier(self, tick_clock, wait_clock):
        nc = self.nc
        probe = nc.sync.nop(nofuse=True)
        wait_clock.add_sem_waits(
            probe.ins, ScopedClock({None: tick_clock.global_clock})
        )
        si = probe.ins.sync_info
        waits = list(si.on_wait) if si else []
        if si:
            si.on_wait = []
        handles = {h.name: h for h in self.sems.allocated().values()}
        for w in waits:
            nc.sync.wait_ge(handles[w.ant_name], w.wait_value)
        nc.sync.drain()
        nc.all_engine_barrier()
        assert self.sems is not None
        popped = nc._tile_sem_poison_stack.pop()
        assert popped is self._sem_poison
        nc.clear_and_free_semaphores(list(self.sems.allocated().values()))
        nc.all_engine_barrier()

    tile.TileContext._drain_and_barrier = _drain_and_barrier
    tile.TileContext._drain_patched = True


_patch_tile_drain()


def _split_excess_waits(nc, cap=1):
    """This walrus build rejects instructions carrying more than ~2 attached
    sem waits. Tile attaches one wait per depended-on logical proc. Rewrite
    every instruction with >cap waits into a chain of single-wait
    InstEventSemaphore ops followed by the instruction keeping `cap` waits."""
    ctr = 0
    for f in nc.m.functions:
        for bb in f.blocks:
            il = bb.instructions
            i = 0
            while i < len(il):
                inst = il[i]
                si = inst.sync_info
                waits = list(si.on_wait) if si else []
                if len(waits) > cap:
                    extra, keep = waits[:-cap], waits[-cap:]
                    evs = []
                    for w in extra:
                        ev = mybir.InstEventSemaphore(
                            name=f"waitsplit-{ctr}", ins=[], outs=[]
                        )
                        ctr += 1
                        ev.engine = inst.engine
                        ev.sync_info = mybir.SyncInfo(on_wait=[w], on_update=[])
                        evs.append(ev)
                    si.on_wait = keep
                    il[i:i] = evs
                    i += len(evs)
                i += 1
    return ctr


def _ternary(w):
    w = np.asarray(w, np.float32)
    return np.where(np.abs(w) < THRESH, 0.0, np.sign(w)).astype(np.float32)


def _pack_dense(m, dtype=np.float16, scale=1.0):
    """[out_f, in_f] f32 -> transposed, tiled [jc, p, ko, j] with
    element = scale*m[jc*128+j, ko*128+p]."""
    import ml_dtypes  # noqa: F401  (np dtype registry)

    of, inf_ = m.shape
    jc, ko = of // P, inf_ // P
    t = np.asarray(m, np.float32).reshape(jc, P, ko, P)  # [jc, j, ko, p]
    t = np.ascontiguousarray(t.transpose(0, 3, 2, 1)) * scale  # [jc, p, ko, j]
    return t.astype(dtype)


def _pack_weight(w, dtype=np.float16, scale=1.0):
    return _pack_dense(_ternary(w), dtype=dtype, scale=scale)


def _pack_x(x_shard):
    """[B, DIM] f32 -> ([p, ko, b] fp16 hi, [p, ko, b] e4m3 of 512*lo)."""
    import ml_dtypes

    b, inf_ = x_shard.shape
    xt = np.ascontiguousarray(
        x_shard.reshape(b, inf_ // P, P).transpose(2, 1, 0)
    ).astype(np.float32)  # [p, ko, b]
    hi = xt.astype(np.float16)
    lo8 = ((xt - hi.astype(np.float32)) * XLO_SCALE).astype(ml_dtypes.float8_e4m3)
    return hi, lo8


def _pack_bias_col(b):
    """[out_f] -> [128, out_f//128] (partition-major)."""
    return np.ascontiguousarray(np.asarray(b, np.float32).reshape(-1, P).T)


def _build_nc():
    nc = bass.Bass()

    xT = nc.declare_dram_parameter("xT", [P, JC_D, B], F16, isOutput=False)
    x8 = nc.declare_dram_parameter("x8", [P, JC_D, B], F8, isOutput=False)
    wf = nc.declare_dram_parameter("wf", [JC_D, P, JC_D, P], F16, isOutput=False)
    wc = nc.declare_dram_parameter("wc", [JC_D, P, JC_D, P], F16, isOutput=False)
    wg = nc.declare_dram_parameter("wg", [JC_D, P, JC_D, P], F16, isOutput=False)
    wf8 = nc.declare_dram_parameter("wf8", [JC_D, P, JC_D, P], F8, isOutput=False)
    wg8 = nc.declare_dram_parameter("wg8", [JC_D, P, JC_D, P], F8, isOutput=False)
    mu = nc.declare_dram_parameter("mu", [JC_H, P, JC_D, P], F16, isOutput=False)
    mg = nc.declare_dram_parameter("mg", [JC_H, P, JC_D, P], F16, isOutput=False)
    wo2 = nc.declare_dram_parameter("wo2", [JC_D, 2, P, JC_H // 2, P], F16, isOutput=False)
    biases = nc.declare_dram_parameter("biases", [P, N_BIAS_COLS], F32, isOutput=False)
    out = nc.declare_dram_parameter("out", [JC_D, P, B], F32, isOutput=True)

    AF = mybir.ActivationFunctionType
    from contextlib import ExitStack

    with tile.TileContext(nc) as tc:
        with (
            tc.tile_pool(name="const", bufs=1) as const,
            tc.tile_pool(name="wpool", bufs=8) as wpool,
            tc.tile_pool(name="psum", bufs=8, space="PSUM") as psum,
        ):
            bias_sb = const.tile([P, N_BIAS_COLS], F32)
            nc.sync.dma_start(out=bias_sb[:], in_=biases[:])

            def bias_ap(col):
                return bias_sb[:, col : col + 1]

            def mm_hi(ps, w_sb, hi_sb, nk, stop=True):
                for ko in range(nk):
                    nc.tensor.matmul(
                        ps, w_sb[:, ko], hi_sb[:, ko],
                        start=(ko == 0), stop=(stop and ko == nk - 1),
                    )

            def mm_lo8(ps, w8_sb, lo8_sb, nk):
                """fp8 DoubleRow lo matmuls (K=256 each) accumulating onto an
                open group. The fp8 weights carry 2^-9 = 1/XLO_SCALE so both
                streams land on the same scale and sum directly in PSUM."""
                for t2 in range(nk // 2):
                    nc.tensor.matmul(
                        ps,
                        w8_sb[:, 2 * t2 : 2 * t2 + 2],
                        lo8_sb[:, 2 * t2 : 2 * t2 + 2],
                        start=False,
                        stop=(t2 == nk // 2 - 1),
                        perf_mode=mybir.MatmulPerfMode.DoubleRow,
                    )

            es_gh = ExitStack()
            gh_pool = es_gh.enter_context(tc.tile_pool(name="gh_pool", bufs=1))
            gh_sb = gh_pool.tile([P, JC_D, B], F16)
            # gu lives on the right-side stack so its (phase 2..3) lifetime
            # can straddle the left-side gh (1..2) lifetime with every pool
            # release still LIFO per side.
            es_gu = ExitStack()

            # ---- phase 1: MLGRU gates; gh = g * ((1-f)*c) -> fp16 ----
            with (
                tc.tile_pool(name="x_pool", bufs=1) as x_pool,
                tc.tile_pool(name="tmp1", bufs=2) as tmp,
            ):
                # first gate weight slab issues before x, in 4 sub-chunks so
                # the first matmuls only wait on ~1/4 of it; x in 4 chunks so
                # matmuls start after ~1/4 of x has landed
                wc0_sb = wpool.tile([P, JC_D, P], F16, tag="w512")
                x_sb = x_pool.tile([P, JC_D, B], F16)
                x8_sb = x_pool.tile([P, JC_D, B], F8)
                # smaller leading chunks so the first matmul's dependencies
                # (wc[0][:, :2] + x[:, :2]) land as early as possible; all
                # x-hi before any x8 (the fp8 lo matmuls run last in their
                # groups, and the first group - the c gate - needs no x8)
                bounds = [0, 2, 4, 8, 12, 16]
                for kc in range(len(bounds) - 1):
                    ks = slice(bounds[kc], bounds[kc + 1])
                    nc.sync.dma_start(out=wc0_sb[:, ks], in_=wc[0][:, ks])
                    nc.sync.dma_start(out=x_sb[:, ks], in_=xT[:, ks])
                for kc in range(JC_D // 4):
                    ks = slice(kc * 4, (kc + 1) * 4)
                    nc.sync.dma_start(out=x8_sb[:, ks], in_=x8[:, ks])

                for jc in range(JC_D):
                    # c first: its stream has no x8/fp8 dependency, giving
                    # the lo-stream DMAs a full group of slack at startup
                    if jc == 0:
                        wc_sb = wc0_sb
                    else:
                        wc_sb = wpool.tile([P, JC_D, P], F16, tag="w512")
                        nc.sync.dma_start(out=wc_sb[:], in_=wc[jc])
                    ps_c = psum.tile([P, B], F32, tag="ps")
                    for ko in range(JC_D):
                        nc.tensor.matmul(
                            ps_c, wc_sb[:, ko], x_sb[:, ko],
                            start=(ko == 0), stop=(ko == JC_D - 1),
                        )

                    # f/g hi streams as fp16, then BOTH fp8 lo blocks
                    # back-to-back: one fp16->fp8 mode transition per jc
                    # instead of two (each transition stalls PE ~0.4us)
                    wf_sb = wpool.tile([P, JC_D, P], F16, tag="w512")
                    nc.sync.dma_start(out=wf_sb[:], in_=wf[jc])
                    wf8_sb = wpool.tile([P, JC_D, P], F8, tag="w256")
                    nc.sync.dma_start(out=wf8_sb[:], in_=wf8[jc])
                    ps_f = psum.tile([P, B], F32, tag="ps")
                    mm_hi(ps_f, wf_sb, x_sb, JC_D, stop=False)

                    wg_sb = wpool.tile([P, JC_D, P], F16, tag="w512")
                    nc.sync.dma_start(out=wg_sb[:], in_=wg[jc])
                    wg8_sb = wpool.tile([P, JC_D, P], F8, tag="w256")
                    nc.sync.dma_start(out=wg8_sb[:], in_=wg8[jc])
                    ps_g = psum.tile([P, B], F32, tag="ps")
                    mm_hi(ps_g, wg_sb, x_sb, JC_D, stop=False)

                    mm_lo8(ps_f, wf8_sb, x8_sb, JC_D)
                    mm_lo8(ps_g, wg8_sb, x8_sb, JC_D)

                    # 1-f = sigmoid(-(t+b)); bias column holds -b_f.
                    # Alternate sigmoid/silu order by jc parity so adjacent
                    # iterations keep the same ACT table loaded.
                    onemf = tmp.tile([P, B], F32, tag="onemf")
                    g_sb = tmp.tile([P, B], F32, tag="g")
                    c_sb = tmp.tile([P, B], F32, tag="c")
                    if jc % 2 == 0:
                        nc.scalar.activation(
                            onemf, ps_f, AF.Sigmoid, bias=bias_ap(COL_NF + jc), scale=-1.0
                        )
                        nc.scalar.activation(g_sb, ps_g, AF.Sigmoid, bias=bias_ap(COL_G + jc))
                        nc.scalar.activation(c_sb, ps_c, AF.Silu, bias=bias_ap(COL_C + jc))
                    else:
                        nc.scalar.activation(c_sb, ps_c, AF.Silu, bias=bias_ap(COL_C + jc))
                        nc.scalar.activation(
                            onemf, ps_f, AF.Sigmoid, bias=bias_ap(COL_NF + jc), scale=-1.0
                        )
                        nc.scalar.activation(g_sb, ps_g, AF.Sigmoid, bias=bias_ap(COL_G + jc))
                    h_sb = tmp.tile([P, B], F32, tag="h")
                    nc.vector.tensor_mul(h_sb, onemf, c_sb)
                    nc.vector.tensor_mul(gh_sb[:, jc], g_sb, h_sb)

            gu_pool = es_gu.enter_context(
                tc.tile_pool(name="gu_pool", bufs=1, side="right")
            )
            gu_sb = gu_pool.tile([P, JC_H, B], F16)
            # prefetch phase 3's first weight slab now so the phase 2->3
            # boundary doesn't stall on a 1MB DMA
            es_w2 = ExitStack()
            wpool2 = es_w2.enter_context(
                tc.tile_pool(name="wpool2", bufs=3, side="right")
            )
            wo2_first = wpool2.tile([P, JC_H // 2, P], F16, tag="w2m")
            nc.sync.dma_start(out=wo2_first[:], in_=wo2[0, 0])

            # ---- phase 2: folded BitGLU pre-acts from gh;
            #      gu = sigmoid(t_g2)*silu(t_u) / 16 -> fp16 ----
            with tc.tile_pool(name="tmp2", bufs=2) as tmp:
                for hc in range(JC_H):
                    mu_sb = wpool.tile([P, JC_D, P], F16, tag="w512")
                    nc.sync.dma_start(out=mu_sb[:], in_=mu[hc])
                    ps_u = psum.tile([P, B], F32, tag="ps")
                    for ko in range(JC_D):
                        nc.tensor.matmul(
                            ps_u, mu_sb[:, ko], gh_sb[:, ko],
                            start=(ko == 0), stop=(ko == JC_D - 1),
                        )

                    mg_sb = wpool.tile([P, JC_D, P], F16, tag="w512")
                    nc.sync.dma_start(out=mg_sb[:], in_=mg[hc])
                    ps_g2 = psum.tile([P, B], F32, tag="ps")
                    for ko in range(JC_D):
                        nc.tensor.matmul(
                            ps_g2, mg_sb[:, ko], gh_sb[:, ko],
                            start=(ko == 0), stop=(ko == JC_D - 1),
                        )

                    u_sb = tmp.tile([P, B], F32, tag="u")
                    g2_sb = tmp.tile([P, B], F32, tag="g2")
                    if hc % 2 == 0:
                        nc.scalar.activation(u_sb, ps_u, AF.Silu, bias=bias_ap(COL_U + hc))
                        nc.scalar.activation(g2_sb, ps_g2, AF.Sigmoid, bias=bias_ap(COL_G2 + hc))
                    else:
                        nc.scalar.activation(g2_sb, ps_g2, AF.Sigmoid, bias=bias_ap(COL_G2 + hc))
                        nc.scalar.activation(u_sb, ps_u, AF.Silu, bias=bias_ap(COL_U + hc))
                    guf = tmp.tile([P, B], F32, tag="guf")
                    nc.vector.tensor_mul(guf, g2_sb, u_sb)
                    nc.vector.tensor_scalar_mul(gu_sb[:, hc], guf, 1.0 / GU_SCALE)
            es_gh.close()

            # ---- phase 3: y = proj_out(gu)*16 + b ----
            with tc.tile_pool(name="outp", bufs=2) as outp:
                for jc in range(JC_D):
                    ps_y = psum.tile([P, B], F32, tag="ps")
                    for half in range(2):
                        if jc == 0 and half == 0:
                            wo2_sb = wo2_first
                        else:
                            wo2_sb = wpool2.tile([P, JC_H // 2, P], F16, tag="w2m")
                            nc.sync.dma_start(out=wo2_sb[:], in_=wo2[jc, half])
                        for kk in range(JC_H // 2):
                            hc = half * (JC_H // 2) + kk
                            nc.tensor.matmul(
                                ps_y,
                                wo2_sb[:, kk],
                                gu_sb[:, hc],
                                start=(hc == 0),
                                stop=(hc == JC_H - 1),
                            )
                    y_sb = outp.tile([P, B], F32, tag="y")
                    nc.vector.tensor_scalar(
                        y_sb, ps_y, GU_SCALE, bias_ap(COL_Y + jc),
                        mybir.AluOpType.mult, mybir.AluOpType.add,
                    )
                    nc.sync.dma_start(out=out[jc], in_=y_sb[:])
            es_w2.close()
            es_gu.close()

    _split_excess_waits(nc)
    return nc


def prep_in_maps(inputs):
    """Build the 8 per-core input maps from the full-size inputs."""
    import ml_dtypes

    x = np.asarray(inputs["x"], np.float32)

    t_wo = _ternary(inputs["out_proj_w"])
    t_wu = _ternary(inputs["proj_u_w"])
    t_wg2 = _ternary(inputs["proj_g_w"])
    # fold out_proj into proj_u / proj_g: integer entries, exact in fp16
    m_u = t_wu @ t_wo  # [HID, DIM]
    m_g = t_wg2 @ t_wo
    b_o = np.asarray(inputs["out_proj_b"], np.float32)
    b_u_eff = t_wu @ b_o + np.asarray(inputs["proj_u_b"], np.float32)
    b_g_eff = t_wg2 @ b_o + np.asarray(inputs["proj_g_b"], np.float32)

    wo2_packed = _pack_weight(inputs["proj_out_w"])  # [JC_D, P, JC_H, P]
    wo2_packed = np.ascontiguousarray(
        wo2_packed.reshape(JC_D, P, 2, JC_H // 2, P).transpose(0, 2, 1, 3, 4)
    )  # [JC_D, 2, P, JC_H//2, P]

    E4 = ml_dtypes.float8_e4m3
    shared = {
        "wf": _pack_weight(inputs["f_gate_w"]),
        "wc": _pack_weight(inputs["c_proj_w"]),
        "wg": _pack_weight(inputs["g_gate_w"]),
        "wf8": _pack_weight(inputs["f_gate_w"], dtype=E4, scale=W8_SCALE),
        "wg8": _pack_weight(inputs["g_gate_w"], dtype=E4, scale=W8_SCALE),
        "mu": _pack_dense(m_u),
        "mg": _pack_dense(m_g),
        "wo2": wo2_packed,
    }
    bias = np.zeros((P, N_BIAS_COLS), np.float32)
    bias[:, COL_NF:COL_NF + JC_D] = _pack_bias_col(-np.asarray(inputs["f_gate_b"]))
    bias[:, COL_C:COL_C + JC_D] = _pack_bias_col(inputs["c_proj_b"])
    bias[:, COL_G:COL_G + JC_D] = _pack_bias_col(inputs["g_gate_b"])
    bias[:, COL_U:COL_U + JC_H] = _pack_bias_col(b_u_eff)
    bias[:, COL_G2:COL_G2 + JC_H] = _pack_bias_col(b_g_eff)
    bias[:, COL_Y:COL_Y + JC_D] = _pack_bias_col(inputs["proj_out_b"])
    shared["biases"] = bias

    in_maps = []
    for core in range(NCORES):
        m = dict(shared)
        m["xT"], m["x8"] = _pack_x(x[core * B : (core + 1) * B])
        in_maps.append(m)
    return in_maps


def gather_output(results):
    """results[i]['out'] is [JC_D, P, B]; assemble full [BATCH, DIM] f32."""
    parts = []
    for core in range(NCORES):
        y = np.asarray(results[core]["out"], np.float32)  # [jc, p, b]
        parts.append(y.reshape(DIM, B).T)  # [b, j]
    return np.ascontiguousarray(np.concatenate(parts, axis=0))


_NC_CACHE = []


def run(inputs, trace=False, **kw):
    from concourse.bass_utils import run_bass_kernel_spmd

    if not _NC_CACHE:
        _NC_CACHE.append(_build_nc())
    nc = _NC_CACHE[0]
    in_maps = prep_in_maps(inputs)
    res = run_bass_kernel_spmd(nc, in_maps, core_ids=list(range(NCORES)), trace=trace, **kw)
    return res


def kernel(**inputs):
    res = run(inputs, trace=False)
    return gather_output(res.results)


# revision 13
# speedup vs baseline: 1.0314x; 1.0314x over previous
"""Trainium2 Bass kernel for nn_BitModel (MLGRU step + BitGLU, ternary weights).

Strategy: pure data-parallel over the 4096 batch dim (512 rows per core,
zero collectives). Weights are ternarized exactly in f32 on the host;
ternary values {-1, 0, +1} are exact in fp16 so device matmuls run at
full 1-cycle/row TensorE rate.

Key restructure vs the fp16 reference pipeline: out_proj is FOLDED into
proj_u / proj_g on the host (M_u = tern(W_u) @ tern(W_o), integer
entries |M| <= ~165, exact in fp16; effective biases folded likewise).
This removes the out_proj matmul stream AND removes one of the fp16
quantization points feeding the proj_g sigmoid, whose saturation-
boundary flips dominate the error budget (t_g2 has std ~1.8e4; a
relative eps on the input becomes ~0.7*sqrt(eps) output error).

Precision scheme (emulated rel err ~1.06e-2, budget 2e-2):
- x feeding f/g gates: fp16 hi + e4m3 lo (x-side flips are the worst).
- x feeding c: fp16 only.
- gh (feeds folded M_u/M_g): fp16.
- gu (feeds proj_out): fp16, pre-scaled by 1/16 for range.
All accumulation f32 in PSUM. Per-core matmul instruction count 4096
(768 fp16 + 256 fp8-DR in phase 1, 2048 + 1024 fp16 in phases 2/3).
"""

import sys

sys.path.insert(0, "/opt/trn_rl_repo")

import numpy as np

import concourse.bass as bass
import concourse.mybir as mybir
import concourse.tile as tile
from concourse.vector_clock import ScopedClock

DIM = 2048
HID = 8192
BATCH = 4096
NCORES = 8
B = BATCH // NCORES  # 512 batch rows per core
P = 128
JC_D = DIM // P  # 16 feature chunks for DIM
JC_H = HID // P  # 64 feature chunks for HID
THRESH = 0.33
GU_SCALE = 16.0  # gu is stored as gu/16 in fp16 to stay inside fp16 range

F16 = mybir.dt.float16
F32 = mybir.dt.float32
F8 = mybir.dt.float8e4  # ml_dtypes.float8_e4m3 (values kept < 240: TRN max)
XLO_SCALE = 512.0  # x_lo is shipped as e4m3(512*x_lo); the fp8 gate weights
W8_SCALE = 2.0 ** -9  # carry the inverse scale (+-2^-9, exact e4m3 subnormals)

# bias column layout in the packed [128, 192] bias tensor
COL_NF = 0  # -f_gate_b (negated: we compute 1-f = sigmoid(-(t+b)))
COL_C = 16
COL_G = 32
COL_U = 48  # effective: tern(W_u) @ out_proj_b + proj_u_b
COL_G2 = 112  # effective: tern(W_g2) @ out_proj_b + proj_g_b
COL_Y = 176
N_BIAS_COLS = 192


def _patch_tile_drain():
    """This walrus build rejects instructions carrying >~2 attached sem
    waits ("Too many sync wait commands") and Tile's kernel-tail drain
    carries one wait per active logical proc. Re-emit those waits as
    standalone wait_ge instructions (1 wait each) before a wait-free
    drain."""
    if getattr(tile.TileContext, "_drain_patched", False):
        return

    def _drain_and_barrier(self, tick_clock, wait_clock):
        nc = self.nc
        probe = nc.sync.nop(nofuse=True)
        wait_clock.add_sem_waits(
            probe.ins, ScopedClock({None: tick_clock.global_clock})
        )
        si = probe.ins.sync_info
        waits = list(si.on_wait) if si else []
        if si:
            si.on_wait = []
        handles = {h.name: h for h in self.sems.allocated().values()}
        for w in waits:
            nc.sync.wait_ge(handles[w.ant_name], w.wait_value)
        nc.sync.drain()
        nc.all_engine_barrier()
        assert self.sems is not None
        popped = nc._tile_sem_poison_stack.pop()
        assert popped is self._sem_poison
        nc.clear_and_free_semaphores(list(self.sems.allocated().values()))
        nc.all_engine_barrier()

    tile.TileContext._drain_and_barrier = _drain_and_barrier
    tile.TileContext._drain_patched = True


_patch_tile_drain()


def _split_excess_waits(nc, cap=1):
    """This walrus build rejects instructions carrying more than ~2 attached
    sem waits. Tile attaches one wait per depended-on logical proc. Rewrite
    every instruction with >cap waits into a chain of single-wait
    InstEventSemaphore ops followed by the instruction keeping `cap` waits."""
    ctr = 0
    for f in nc.m.functions:
        for bb in f.blocks:
            il = bb.instructions
            i = 0
            while i < len(il):
                inst = il[i]
                si = inst.sync_info
                waits = list(si.on_wait) if si else []
                if len(waits) > cap:
                    extra, keep = waits[:-cap], waits[-cap:]
                    evs = []
                    for w in extra:
                        ev = mybir.InstEventSemaphore(
                            name=f"waitsplit-{ctr}", ins=[], outs=[]
                        )
                        ctr += 1
                        ev.engine = inst.engine
                        ev.sync_info = mybir.SyncInfo(on_wait=[w], on_update=[])
                        evs.append(ev)
                    si.on_wait = keep
                    il[i:i] = evs
                    i += len(evs)
                i += 1
    return ctr


def _ternary(w):
    w = np.asarray(w, np.float32)
    return np.where(np.abs(w) < THRESH, 0.0, np.sign(w)).astype(np.float32)


def _pack_dense(m, dtype=np.float16, scale=1.0):
    """[out_f, in_f] f32 -> transposed, tiled [jc, p, ko, j] with
    element = scale*m[jc*128+j, ko*128+p]."""
    import ml_dtypes  # noqa: F401  (np dtype registry)

    of, inf_ = m.shape
    jc, ko = of // P, inf_ // P
    t = np.asarray(m, np.float32).reshape(jc, P, ko, P)  # [jc, j, ko, p]
    t = np.ascontiguousarray(t.transpose(0, 3, 2, 1)) * scale  # [jc, p, ko, j]
    return t.astype(dtype)


def _pack_weight(w, dtype=np.float16, scale=1.0):
    return _pack_dense(_ternary(w), dtype=dtype, scale=scale)


def _pack_x(x_shard):
    """[B, DIM] f32 -> ([p, ko, b] fp16 hi, [p, ko, b] e4m3 of 512*lo)."""
    import ml_dtypes

    b, inf_ = x_shard.shape
    xt = np.ascontiguousarray(
        x_shard.reshape(b, inf_ // P, P).transpose(2, 1, 0)
    ).astype(np.float32)  # [p, ko, b]
    hi = xt.astype(np.float16)
    lo8 = ((xt - hi.astype(np.float32)) * XLO_SCALE).astype(ml_dtypes.float8_e4m3)
    return hi, lo8


def _pack_bias_col(b):
    """[out_f] -> [128, out_f//128] (partition-major)."""
    return np.ascontiguousarray(np.asarray(b, np.float32).reshape(-1, P).T)


def _build_nc():
    nc = bass.Bass()

    xT = nc.declare_dram_parameter("xT", [P, JC_D, B], F16, isOutput=False)
    x8 = nc.declare_dram_parameter("x8", [P, JC_D, B], F8, isOutput=False)
    wf = nc.declare_dram_parameter("wf", [JC_D, P, JC_D, P], F16, isOutput=False)
    wc = nc.declare_dram_parameter("wc", [JC_D, P, JC_D, P], F16, isOutput=False)
    wg = nc.declare_dram_parameter("wg", [JC_D, P, JC_D, P], F16, isOutput=False)
    wg8 = nc.declare_dram_parameter("wg8", [JC_D, P, JC_D, P], F8, isOutput=False)
    mu = nc.declare_dram_parameter("mu", [JC_H, P, JC_D, P], F16, isOutput=False)
    mg = nc.declare_dram_parameter("mg", [JC_H, P, JC_D, P], F16, isOutput=False)
    wo2 = nc.declare_dram_parameter("wo2", [JC_D, 2, P, JC_H // 2, P], F16, isOutput=False)
    biases = nc.declare_dram_parameter("biases", [P, N_BIAS_COLS], F32, isOutput=False)
    out = nc.declare_dram_parameter("out", [JC_D, P, B], F32, isOutput=True)

    AF = mybir.ActivationFunctionType
    from contextlib import ExitStack

    with tile.TileContext(nc) as tc:
        with (
            tc.tile_pool(name="const", bufs=1) as const,
            tc.tile_pool(name="wpool", bufs=8) as wpool,
            tc.tile_pool(name="psum", bufs=8, space="PSUM") as psum,
        ):
            bias_sb = const.tile([P, N_BIAS_COLS], F32)
            nc.sync.dma_start(out=bias_sb[:], in_=biases[:])

            def bias_ap(col):
                return bias_sb[:, col : col + 1]

            def mm_hi(ps, w_sb, hi_sb, nk, stop=True):
                for ko in range(nk):
                    nc.tensor.matmul(
                        ps, w_sb[:, ko], hi_sb[:, ko],
                        start=(ko == 0), stop=(stop and ko == nk - 1),
                    )

            def mm_lo8(ps, w8_sb, lo8_sb, nk):
                """fp8 DoubleRow lo matmuls (K=256 each) accumulating onto an
                open group. The fp8 weights carry 2^-9 = 1/XLO_SCALE so both
                streams land on the same scale and sum directly in PSUM."""
                for t2 in range(nk // 2):
                    nc.tensor.matmul(
                        ps,
                        w8_sb[:, 2 * t2 : 2 * t2 + 2],
                        lo8_sb[:, 2 * t2 : 2 * t2 + 2],
                        start=False,
                        stop=(t2 == nk // 2 - 1),
                        perf_mode=mybir.MatmulPerfMode.DoubleRow,
                    )

            es_gh = ExitStack()
            gh_pool = es_gh.enter_context(tc.tile_pool(name="gh_pool", bufs=1))
            gh_sb = gh_pool.tile([P, JC_D, B], F16)
            # gu lives on the right-side stack so its (phase 2..3) lifetime
            # can straddle the left-side gh (1..2) lifetime with every pool
            # release still LIFO per side.
            es_gu = ExitStack()

            # ---- phase 1: MLGRU gates; gh = g * ((1-f)*c) -> fp16 ----
            with (
                tc.tile_pool(name="x_pool", bufs=1) as x_pool,
                tc.tile_pool(name="tmp1", bufs=2) as tmp,
            ):
                # first gate weight slab issues before x, in 4 sub-chunks so
                # the first matmuls only wait on ~1/4 of it; x in 4 chunks so
                # matmuls start after ~1/4 of x has landed
                wc0_sb = wpool.tile([P, JC_D, P], F16, tag="w512")
                x_sb = x_pool.tile([P, JC_D, B], F16)
                x8_sb = x_pool.tile([P, JC_D, B], F8)
                # smaller leading chunks so the first matmul's dependencies
                # (wc[0][:, :2] + x[:, :2]) land as early as possible; all
                # x-hi before any x8 (the fp8 lo matmuls run last in their
                # groups, and the first group - the c gate - needs no x8)
                bounds = [0, 2, 4, 8, 12, 16]
                for kc in range(len(bounds) - 1):
                    ks = slice(bounds[kc], bounds[kc + 1])
                    nc.sync.dma_start(out=wc0_sb[:, ks], in_=wc[0][:, ks])
                    nc.sync.dma_start(out=x_sb[:, ks], in_=xT[:, ks])
                for kc in range(JC_D // 4):
                    ks = slice(kc * 4, (kc + 1) * 4)
                    nc.sync.dma_start(out=x8_sb[:, ks], in_=x8[:, ks])

                for jc in range(JC_D):
                    # c first: its stream has no x8/fp8 dependency, giving
                    # the lo-stream DMAs a full group of slack at startup
                    if jc == 0:
                        wc_sb = wc0_sb
                    else:
                        wc_sb = wpool.tile([P, JC_D, P], F16, tag="w512")
                        nc.sync.dma_start(out=wc_sb[:], in_=wc[jc])
                    ps_c = psum.tile([P, B], F32, tag="ps")
                    for ko in range(JC_D):
                        nc.tensor.matmul(
                            ps_c, wc_sb[:, ko], x_sb[:, ko],
                            start=(ko == 0), stop=(ko == JC_D - 1),
                        )

                    # f runs on fp16 x only (its flip error fits the budget);
                    # g keeps the fp8 lo refinement, placed last so there is
                    # one fp16->fp8 mode transition per jc
                    wf_sb = wpool.tile([P, JC_D, P], F16, tag="w512")
                    nc.sync.dma_start(out=wf_sb[:], in_=wf[jc])
                    ps_f = psum.tile([P, B], F32, tag="ps")
                    mm_hi(ps_f, wf_sb, x_sb, JC_D)

                    wg_sb = wpool.tile([P, JC_D, P], F16, tag="w512")
                    nc.sync.dma_start(out=wg_sb[:], in_=wg[jc])
                    wg8_sb = wpool.tile([P, JC_D, P], F8, tag="w256")
                    nc.sync.dma_start(out=wg8_sb[:], in_=wg8[jc])
                    ps_g = psum.tile([P, B], F32, tag="ps")
                    mm_hi(ps_g, wg_sb, x_sb, JC_D, stop=False)
                    mm_lo8(ps_g, wg8_sb, x8_sb, JC_D)

                    # 1-f = sigmoid(-(t+b)); bias column holds -b_f.
                    # Alternate sigmoid/silu order by jc parity so adjacent
                    # iterations keep the same ACT table loaded.
                    onemf = tmp.tile([P, B], F32, tag="onemf")
                    g_sb = tmp.tile([P, B], F32, tag="g")
                    c_sb = tmp.tile([P, B], F32, tag="c")
                    if jc % 2 == 0:
                        nc.scalar.activation(
                            onemf, ps_f, AF.Sigmoid, bias=bias_ap(COL_NF + jc), scale=-1.0
                        )
                        nc.scalar.activation(g_sb, ps_g, AF.Sigmoid, bias=bias_ap(COL_G + jc))
                        nc.scalar.activation(c_sb, ps_c, AF.Silu, bias=bias_ap(COL_C + jc))
                    else:
                        nc.scalar.activation(c_sb, ps_c, AF.Silu, bias=bias_ap(COL_C + jc))
                        nc.scalar.activation(
                            onemf, ps_f, AF.Sigmoid, bias=bias_ap(COL_NF + jc), scale=-1.0
                        )
                        nc.scalar.activation(g_sb, ps_g, AF.Sigmoid, bias=bias_ap(COL_G + jc))
                    h_sb = tmp.tile([P, B], F32, tag="h")
                    nc.vector.tensor_mul(h_sb, onemf, c_sb)
                    nc.vector.tensor_mul(gh_sb[:, jc], g_sb, h_sb)

            gu_pool = es_gu.enter_context(
                tc.tile_pool(name="gu_pool", bufs=1, side="right")
            )
            gu_sb = gu_pool.tile([P, JC_H, B], F16)
            # prefetch phase 3's first weight slab now so the phase 2->3
            # boundary doesn't stall on a 1MB DMA
            es_w2 = ExitStack()
            wpool2 = es_w2.enter_context(
                tc.tile_pool(name="wpool2", bufs=3, side="right")
            )
            wo2_first = wpool2.tile([P, JC_H // 2, P], F16, tag="w2m")
            nc.sync.dma_start(out=wo2_first[:], in_=wo2[0, 0])

            # ---- phase 2: folded BitGLU pre-acts from gh;
            #      gu = sigmoid(t_g2)*silu(t_u) / 16 -> fp16 ----
            with tc.tile_pool(name="tmp2", bufs=2) as tmp:
                for hc in range(JC_H):
                    mu_sb = wpool.tile([P, JC_D, P], F16, tag="w512")
                    nc.sync.dma_start(out=mu_sb[:], in_=mu[hc])
                    ps_u = psum.tile([P, B], F32, tag="ps")
                    for ko in range(JC_D):
                        nc.tensor.matmul(
                            ps_u, mu_sb[:, ko], gh_sb[:, ko],
                            start=(ko == 0), stop=(ko == JC_D - 1),
                        )

                    mg_sb = wpool.tile([P, JC_D, P], F16, tag="w512")
                    nc.sync.dma_start(out=mg_sb[:], in_=mg[hc])
                    ps_g2 = psum.tile([P, B], F32, tag="ps")
                    for ko in range(JC_D):
                        nc.tensor.matmul(
                            ps_g2, mg_sb[:, ko], gh_sb[:, ko],
                            start=(ko == 0), stop=(ko == JC_D - 1),
                        )

                    u_sb = tmp.tile([P, B], F32, tag="u")
                    g2_sb = tmp.tile([P, B], F32, tag="g2")
                    if hc % 2 == 0:
                        nc.scalar.activation(u_sb, ps_u, AF.Silu, bias=bias_ap(COL_U + hc))
                        nc.scalar.activation(g2_sb, ps_g2, AF.Sigmoid, bias=bias_ap(COL_G2 + hc))
                    else:
                        nc.scalar.activation(g2_sb, ps_g2, AF.Sigmoid, bias=bias_ap(COL_G2 + hc))
                        nc.scalar.activation(u_sb, ps_u, AF.Silu, bias=bias_ap(COL_U + hc))
                    guf = tmp.tile([P, B], F32, tag="guf")
                    nc.vector.tensor_mul(guf, g2_sb, u_sb)
                    nc.vector.tensor_scalar_mul(gu_sb[:, hc], guf, 1.0 / GU_SCALE)
            es_gh.close()

            # ---- phase 3: y = proj_out(gu)*16 + b ----
            with tc.tile_pool(name="outp", bufs=2) as outp:
                for jc in range(JC_D):
                    ps_y = psum.tile([P, B], F32, tag="ps")
                    for half in range(2):
                        if jc == 0 and half == 0:
                            wo2_sb = wo2_first
                        else:
                            wo2_sb = wpool2.tile([P, JC_H // 2, P], F16, tag="w2m")
                            nc.sync.dma_start(out=wo2_sb[:], in_=wo2[jc, half])
                        for kk in range(JC_H // 2):
                            hc = half * (JC_H // 2) + kk
                            nc.tensor.matmul(
                                ps_y,
                                wo2_sb[:, kk],
                                gu_sb[:, hc],
                                start=(hc == 0),
                                stop=(hc == JC_H - 1),
                            )
                    y_sb = outp.tile([P, B], F32, tag="y")
                    nc.vector.tensor_scalar(
                        y_sb, ps_y, GU_SCALE, bias_ap(COL_Y + jc),
                        mybir.AluOpType.mult, mybir.AluOpType.add,
                    )
                    nc.sync.dma_start(out=out[jc], in_=y_sb[:])
            es_w2.close()
            es_gu.close()

    _split_excess_waits(nc)
    return nc


def prep_in_maps(inputs):
    """Build the 8 per-core input maps from the full-size inputs."""
    import ml_dtypes

    x = np.asarray(inputs["x"], np.float32)

    t_wo = _ternary(inputs["out_proj_w"])
    t_wu = _ternary(inputs["proj_u_w"])
    t_wg2 = _ternary(inputs["proj_g_w"])
    # fold out_proj into proj_u / proj_g: integer entries, exact in fp16
    m_u = t_wu @ t_wo  # [HID, DIM]
    m_g = t_wg2 @ t_wo
    b_o = np.asarray(inputs["out_proj_b"], np.float32)
    b_u_eff = t_wu @ b_o + np.asarray(inputs["proj_u_b"], np.float32)
    b_g_eff = t_wg2 @ b_o + np.asarray(inputs["proj_g_b"], np.float32)

    wo2_packed = _pack_weight(inputs["proj_out_w"])  # [JC_D, P, JC_H, P]
    wo2_packed = np.ascontiguousarray(
        wo2_packed.reshape(JC_D, P, 2, JC_H // 2, P).transpose(0, 2, 1, 3, 4)
    )  # [JC_D, 2, P, JC_H//2, P]

    E4 = ml_dtypes.float8_e4m3
    shared = {
        "wf": _pack_weight(inputs["f_gate_w"]),
        "wc": _pack_weight(inputs["c_proj_w"]),
        "wg": _pack_weight(inputs["g_gate_w"]),
        "wg8": _pack_weight(inputs["g_gate_w"], dtype=E4, scale=W8_SCALE),
        "mu": _pack_dense(m_u),
        "mg": _pack_dense(m_g),
        "wo2": wo2_packed,
    }
    bias = np.zeros((P, N_BIAS_COLS), np.float32)
    bias[:, COL_NF:COL_NF + JC_D] = _pack_bias_col(-np.asarray(inputs["f_gate_b"]))
    bias[:, COL_C:COL_C + JC_D] = _pack_bias_col(inputs["c_proj_b"])
    bias[:, COL_G:COL_G + JC_D] = _pack_bias_col(inputs["g_gate_b"])
    bias[:, COL_U:COL_U + JC_H] = _pack_bias_col(b_u_eff)
    bias[:, COL_G2:COL_G2 + JC_H] = _pack_bias_col(b_g_eff)
    bias[:, COL_Y:COL_Y + JC_D] = _pack_bias_col(inputs["proj_out_b"])
    shared["biases"] = bias

    in_maps = []
    for core in range(NCORES):
        m = dict(shared)
        m["xT"], m["x8"] = _pack_x(x[core * B : (core + 1) * B])
        in_maps.append(m)
    return in_maps


def gather_output(results):
    """results[i]['out'] is [JC_D, P, B]; assemble full [BATCH, DIM] f32."""
    parts = []
    for core in range(NCORES):
        y = np.asarray(results[core]["out"], np.float32)  # [jc, p, b]
        parts.append(y.reshape(DIM, B).T)  # [b, j]
    return np.ascontiguousarray(np.concatenate(parts, axis=0))


_NC_CACHE = []


def run(inputs, trace=False, **kw):
    from concourse.bass_utils import run_bass_kernel_spmd

    if not _NC_CACHE:
        _NC_CACHE.append(_build_nc())
    nc = _NC_CACHE[0]
    in_maps = prep_in_maps(inputs)
    res = run_bass_kernel_spmd(nc, in_maps, core_ids=list(range(NCORES)), trace=trace, **kw)
    return res


def kernel(**inputs):
    res = run(inputs, trace=False)
    return gather_output(res.results)


# revision 16
# speedup vs baseline: 1.0377x; 1.0061x over previous
"""Trainium2 Bass kernel for nn_BitModel (MLGRU step + BitGLU, ternary weights).

Strategy: pure data-parallel over the 4096 batch dim (512 rows per core,
zero collectives). Weights are ternarized exactly in f32 on the host;
ternary values {-1, 0, +1} are exact in fp16 so device matmuls run at
full 1-cycle/row TensorE rate.

Key restructure vs the fp16 reference pipeline: out_proj is FOLDED into
proj_u / proj_g on the host (M_u = tern(W_u) @ tern(W_o), integer
entries |M| <= ~165, exact in fp16; effective biases folded likewise).
This removes the out_proj matmul stream AND removes one of the fp16
quantization points feeding the proj_g sigmoid, whose saturation-
boundary flips dominate the error budget (t_g2 has std ~1.8e4; a
relative eps on the input becomes ~0.7*sqrt(eps) output error).

Precision scheme (emulated rel err ~1.06e-2, budget 2e-2):
- x feeding f/g gates: fp16 hi + e4m3 lo (x-side flips are the worst).
- x feeding c: fp16 only.
- gh (feeds folded M_u/M_g): fp16.
- gu (feeds proj_out): fp16, pre-scaled by 1/16 for range.
All accumulation f32 in PSUM. Per-core matmul instruction count 4096
(768 fp16 + 256 fp8-DR in phase 1, 2048 + 1024 fp16 in phases 2/3).
"""

import sys

sys.path.insert(0, "/opt/trn_rl_repo")

import numpy as np

import concourse.bass as bass
import concourse.mybir as mybir
import concourse.tile as tile
from concourse.vector_clock import ScopedClock

DIM = 2048
HID = 8192
BATCH = 4096
NCORES = 8
B = BATCH // NCORES  # 512 batch rows per core
P = 128
JC_D = DIM // P  # 16 feature chunks for DIM
JC_H = HID // P  # 64 feature chunks for HID
THRESH = 0.33
GU_SCALE = 16.0  # gu is stored as gu/16 in fp16 to stay inside fp16 range

F16 = mybir.dt.float16
F32 = mybir.dt.float32
F8 = mybir.dt.float8e4  # ml_dtypes.float8_e4m3 (values kept < 240: TRN max)
XLO_SCALE = 512.0  # x_lo is shipped as e4m3(512*x_lo); the fp8 gate weights
W8_SCALE = 2.0 ** -9  # carry the inverse scale (+-2^-9, exact e4m3 subnormals)

# bias column layout in the packed [128, 192] bias tensor
COL_NF = 0  # -f_gate_b (negated: we compute 1-f = sigmoid(-(t+b)))
COL_C = 16
COL_G = 32
COL_U = 48  # effective: tern(W_u) @ out_proj_b + proj_u_b
COL_G2 = 112  # effective: tern(W_g2) @ out_proj_b + proj_g_b
COL_Y = 176
N_BIAS_COLS = 192


def _patch_tile_drain():
    """This walrus build rejects instructions carrying >~2 attached sem
    waits ("Too many sync wait commands") and Tile's kernel-tail drain
    carries one wait per active logical proc. Re-emit those waits as
    standalone wait_ge instructions (1 wait each) before a wait-free
    drain."""
    if getattr(tile.TileContext, "_drain_patched", False):
        return

    def _drain_and_barrier(self, tick_clock, wait_clock):
        nc = self.nc
        probe = nc.sync.nop(nofuse=True)
        wait_clock.add_sem_waits(
            probe.ins, ScopedClock({None: tick_clock.global_clock})
        )
        si = probe.ins.sync_info
        waits = list(si.on_wait) if si else []
        if si:
            si.on_wait = []
        handles = {h.name: h for h in self.sems.allocated().values()}
        for w in waits:
            nc.sync.wait_ge(handles[w.ant_name], w.wait_value)
        nc.sync.drain()
        nc.all_engine_barrier()
        assert self.sems is not None
        popped = nc._tile_sem_poison_stack.pop()
        assert popped is self._sem_poison
        nc.clear_and_free_semaphores(list(self.sems.allocated().values()))
        nc.all_engine_barrier()

    tile.TileContext._drain_and_barrier = _drain_and_barrier
    tile.TileContext._drain_patched = True


_patch_tile_drain()


def _split_excess_waits(nc, cap=1):
    """This walrus build rejects instructions carrying more than ~2 attached
    sem waits. Tile attaches one wait per depended-on logical proc. Rewrite
    every instruction with >cap waits into a chain of single-wait
    InstEventSemaphore ops followed by the instruction keeping `cap` waits."""
    ctr = 0
    for f in nc.m.functions:
        for bb in f.blocks:
            il = bb.instructions
            i = 0
            while i < len(il):
                inst = il[i]
                si = inst.sync_info
                waits = list(si.on_wait) if si else []
                if len(waits) > cap:
                    extra, keep = waits[:-cap], waits[-cap:]
                    evs = []
                    for w in extra:
                        ev = mybir.InstEventSemaphore(
                            name=f"waitsplit-{ctr}", ins=[], outs=[]
                        )
                        ctr += 1
                        ev.engine = inst.engine
                        ev.sync_info = mybir.SyncInfo(on_wait=[w], on_update=[])
                        evs.append(ev)
                    si.on_wait = keep
                    il[i:i] = evs
                    i += len(evs)
                i += 1
    return ctr


def _ternary(w):
    w = np.asarray(w, np.float32)
    return np.where(np.abs(w) < THRESH, 0.0, np.sign(w)).astype(np.float32)


def _pack_dense(m, dtype=np.float16, scale=1.0):
    """[out_f, in_f] f32 -> transposed, tiled [jc, p, ko, j] with
    element = scale*m[jc*128+j, ko*128+p]."""
    import ml_dtypes  # noqa: F401  (np dtype registry)

    of, inf_ = m.shape
    jc, ko = of // P, inf_ // P
    t = np.asarray(m, np.float32).reshape(jc, P, ko, P)  # [jc, j, ko, p]
    t = np.ascontiguousarray(t.transpose(0, 3, 2, 1)) * scale  # [jc, p, ko, j]
    return t.astype(dtype)


def _pack_weight(w, dtype=np.float16, scale=1.0):
    return _pack_dense(_ternary(w), dtype=dtype, scale=scale)


def _pack_x(x_shard):
    """[B, DIM] f32 -> ([p, ko, b] fp16 hi, [p, ko, b] e4m3 of 512*lo)."""
    import ml_dtypes

    b, inf_ = x_shard.shape
    xt = np.ascontiguousarray(
        x_shard.reshape(b, inf_ // P, P).transpose(2, 1, 0)
    ).astype(np.float32)  # [p, ko, b]
    hi = xt.astype(np.float16)
    lo8 = ((xt - hi.astype(np.float32)) * XLO_SCALE).astype(ml_dtypes.float8_e4m3)
    return hi, lo8


def _pack_bias_col(b):
    """[out_f] -> [128, out_f//128] (partition-major)."""
    return np.ascontiguousarray(np.asarray(b, np.float32).reshape(-1, P).T)


def _build_nc():
    nc = bass.Bass()

    xT = nc.declare_dram_parameter("xT", [P, JC_D, B], F16, isOutput=False)
    x8 = nc.declare_dram_parameter("x8", [P, JC_D, B], F8, isOutput=False)
    wf = nc.declare_dram_parameter("wf", [JC_D, P, JC_D, P], F16, isOutput=False)
    wc = nc.declare_dram_parameter("wc", [JC_D, P, JC_D, P], F16, isOutput=False)
    wg = nc.declare_dram_parameter("wg", [JC_D, P, JC_D, P], F16, isOutput=False)
    wg8 = nc.declare_dram_parameter("wg8", [JC_D, P, JC_D, P], F8, isOutput=False)
    mu = nc.declare_dram_parameter("mu", [JC_H, P, JC_D, P], F16, isOutput=False)
    mg = nc.declare_dram_parameter("mg", [JC_H, P, JC_D, P], F16, isOutput=False)
    wo2 = nc.declare_dram_parameter("wo2", [JC_D, 2, P, JC_H // 2, P], F16, isOutput=False)
    biases = nc.declare_dram_parameter("biases", [P, N_BIAS_COLS], F32, isOutput=False)
    out = nc.declare_dram_parameter("out", [JC_D, P, B], F32, isOutput=True)

    AF = mybir.ActivationFunctionType
    from contextlib import ExitStack

    with tile.TileContext(nc) as tc:
        with (
            tc.tile_pool(name="const", bufs=1) as const,
            tc.tile_pool(name="wpool", bufs=8) as wpool,
            tc.tile_pool(name="psum", bufs=8, space="PSUM") as psum,
        ):
            bias_sb = const.tile([P, N_BIAS_COLS], F32)
            nc.sync.dma_start(out=bias_sb[:], in_=biases[:])

            def bias_ap(col):
                return bias_sb[:, col : col + 1]

            def mm_hi(ps, w_sb, hi_sb, nk, stop=True):
                for ko in range(nk):
                    nc.tensor.matmul(
                        ps, w_sb[:, ko], hi_sb[:, ko],
                        start=(ko == 0), stop=(stop and ko == nk - 1),
                    )

            def mm_lo8(ps, w8_sb, lo8_sb, nk):
                """fp8 DoubleRow lo matmuls (K=256 each) accumulating onto an
                open group. The fp8 weights carry 2^-9 = 1/XLO_SCALE so both
                streams land on the same scale and sum directly in PSUM."""
                for t2 in range(nk // 2):
                    nc.tensor.matmul(
                        ps,
                        w8_sb[:, 2 * t2 : 2 * t2 + 2],
                        lo8_sb[:, 2 * t2 : 2 * t2 + 2],
                        start=False,
                        stop=(t2 == nk // 2 - 1),
                        perf_mode=mybir.MatmulPerfMode.DoubleRow,
                    )

            es_gh = ExitStack()
            gh_pool = es_gh.enter_context(tc.tile_pool(name="gh_pool", bufs=1))
            gh_sb = gh_pool.tile([P, JC_D, B], F16)
            # gu lives on the right-side stack so its (phase 2..3) lifetime
            # can straddle the left-side gh (1..2) lifetime with every pool
            # release still LIFO per side.
            es_gu = ExitStack()

            # ---- phase 1: MLGRU gates; gh = g * ((1-f)*c) -> fp16 ----
            with (
                tc.tile_pool(name="x_pool", bufs=1) as x_pool,
                tc.tile_pool(name="tmp1", bufs=2) as tmp,
            ):
                # first gate weight slab issues before x, in 4 sub-chunks so
                # the first matmuls only wait on ~1/4 of it; x in 4 chunks so
                # matmuls start after ~1/4 of x has landed
                wc0_sb = wpool.tile([P, JC_D, P], F16, tag="w512")
                wf0_sb = wpool.tile([P, JC_D, P], F16, tag="w512")
                wg0_sb = wpool.tile([P, JC_D, P], F16, tag="w512")
                wg80_sb = wpool.tile([P, JC_D, P], F8, tag="w256")
                x_sb = x_pool.tile([P, JC_D, B], F16)
                x8_sb = x_pool.tile([P, JC_D, B], F8)
                # startup issue order, tuned: each dma_start costs ~0.5us of
                # sync-queue issue time, so keep the critical chain (first c
                # matmuls: wc[0][:, :4] + x[:, :4]) to 2 issues, then jc=0's
                # other weight slabs, then the fp8 x-lo (needed latest)
                nc.sync.dma_start(out=wc0_sb[:, 0:4], in_=wc[0][:, 0:4])
                nc.sync.dma_start(out=x_sb[:, 0:4], in_=xT[:, 0:4])
                nc.sync.dma_start(out=wc0_sb[:, 4:16], in_=wc[0][:, 4:16])
                nc.sync.dma_start(out=x_sb[:, 4:10], in_=xT[:, 4:10])
                nc.sync.dma_start(out=x_sb[:, 10:16], in_=xT[:, 10:16])
                nc.sync.dma_start(out=wf0_sb[:], in_=wf[0])
                nc.sync.dma_start(out=wg0_sb[:], in_=wg[0])
                nc.sync.dma_start(out=wg80_sb[:], in_=wg8[0])
                nc.sync.dma_start(out=x8_sb[:, 0:8], in_=x8[:, 0:8])
                nc.sync.dma_start(out=x8_sb[:, 8:16], in_=x8[:, 8:16])

                # process jc in pairs: the two g-lo fp8 blocks of a pair run
                # back-to-back, so there is one fp16->fp8 mode transition per
                # TWO jc (each transition stalls the PE ~0.4us). 6 PSUM banks
                # live per pair (c/f/g x 2), within the 8-bank ring.
                for jp in range(JC_D // 2):
                    ps_c = [None, None]
                    ps_f = [None, None]
                    ps_g = [None, None]
                    wg8_sb = [None, None]
                    for s in range(2):
                        jc = 2 * jp + s
                        # c first: its stream has no x8/fp8 dependency,
                        # giving the lo-stream DMAs slack at startup
                        if jc == 0:
                            wc_sb = wc0_sb
                        else:
                            wc_sb = wpool.tile([P, JC_D, P], F16, tag="w512")
                            nc.sync.dma_start(out=wc_sb[:], in_=wc[jc])
                        ps_c[s] = psum.tile([P, B], F32, tag="ps", name=f"ps_c{s}")
                        mm_hi(ps_c[s], wc_sb, x_sb, JC_D)

                        # f runs on fp16 x only (its flip error fits the
                        # error budget); g keeps the fp8 lo refinement
                        if jc == 0:
                            wf_sb = wf0_sb
                        else:
                            wf_sb = wpool.tile([P, JC_D, P], F16, tag="w512")
                            nc.sync.dma_start(out=wf_sb[:], in_=wf[jc])
                        ps_f[s] = psum.tile([P, B], F32, tag="ps", name=f"ps_f{s}")
                        mm_hi(ps_f[s], wf_sb, x_sb, JC_D)

                        if jc == 0:
                            wg_sb = wg0_sb
                            wg8_sb[s] = wg80_sb
                        else:
                            wg_sb = wpool.tile([P, JC_D, P], F16, tag="w512")
                            nc.sync.dma_start(out=wg_sb[:], in_=wg[jc])
                            wg8_sb[s] = wpool.tile([P, JC_D, P], F8, tag="w256", name=f"wg8_{s}")
                            nc.sync.dma_start(out=wg8_sb[s][:], in_=wg8[jc])
                        ps_g[s] = psum.tile([P, B], F32, tag="ps", name=f"ps_g{s}")
                        mm_hi(ps_g[s], wg_sb, x_sb, JC_D, stop=False)

                    mm_lo8(ps_g[0], wg8_sb[0], x8_sb, JC_D)
                    mm_lo8(ps_g[1], wg8_sb[1], x8_sb, JC_D)

                    for s in range(2):
                        jc = 2 * jp + s
                        # 1-f = sigmoid(-(t+b)); bias column holds -b_f.
                        # Alternate sigmoid/silu order by jc parity so
                        # adjacent iterations keep the same ACT table loaded.
                        onemf = tmp.tile([P, B], F32, tag="onemf")
                        g_sb = tmp.tile([P, B], F32, tag="g")
                        c_sb = tmp.tile([P, B], F32, tag="c")
                        if jc % 2 == 0:
                            nc.scalar.activation(
                                onemf, ps_f[s], AF.Sigmoid, bias=bias_ap(COL_NF + jc), scale=-1.0
                            )
                            nc.scalar.activation(g_sb, ps_g[s], AF.Sigmoid, bias=bias_ap(COL_G + jc))
                            nc.scalar.activation(c_sb, ps_c[s], AF.Silu, bias=bias_ap(COL_C + jc))
                        else:
                            nc.scalar.activation(c_sb, ps_c[s], AF.Silu, bias=bias_ap(COL_C + jc))
                            nc.scalar.activation(
                                onemf, ps_f[s], AF.Sigmoid, bias=bias_ap(COL_NF + jc), scale=-1.0
                            )
                            nc.scalar.activation(g_sb, ps_g[s], AF.Sigmoid, bias=bias_ap(COL_G + jc))
                        h_sb = tmp.tile([P, B], F32, tag="h")
                        nc.vector.tensor_mul(h_sb, onemf, c_sb)
                        nc.vector.tensor_mul(gh_sb[:, jc], g_sb, h_sb)

            gu_pool = es_gu.enter_context(
                tc.tile_pool(name="gu_pool", bufs=1, side="right")
            )
            gu_sb = gu_pool.tile([P, JC_H, B], F16)
            # prefetch phase 3's first weight slab now so the phase 2->3
            # boundary doesn't stall on a 1MB DMA
            es_w2 = ExitStack()
            wpool2 = es_w2.enter_context(
                tc.tile_pool(name="wpool2", bufs=3, side="right")
            )
            wo2_first = wpool2.tile([P, JC_H // 2, P], F16, tag="w2m")
            nc.sync.dma_start(out=wo2_first[:], in_=wo2[0, 0])

            # ---- phase 2: folded BitGLU pre-acts from gh;
            #      gu = sigmoid(t_g2)*silu(t_u) / 16 -> fp16 ----
            with tc.tile_pool(name="tmp2", bufs=2) as tmp:
                for hc in range(JC_H):
                    mu_sb = wpool.tile([P, JC_D, P], F16, tag="w512")
                    nc.sync.dma_start(out=mu_sb[:], in_=mu[hc])
                    ps_u = psum.tile([P, B], F32, tag="ps")
                    for ko in range(JC_D):
                        nc.tensor.matmul(
                            ps_u, mu_sb[:, ko], gh_sb[:, ko],
                            start=(ko == 0), stop=(ko == JC_D - 1),
                        )

                    mg_sb = wpool.tile([P, JC_D, P], F16, tag="w512")
                    nc.sync.dma_start(out=mg_sb[:], in_=mg[hc])
                    ps_g2 = psum.tile([P, B], F32, tag="ps")
                    for ko in range(JC_D):
                        nc.tensor.matmul(
                            ps_g2, mg_sb[:, ko], gh_sb[:, ko],
                            start=(ko == 0), stop=(ko == JC_D - 1),
                        )

                    u_sb = tmp.tile([P, B], F32, tag="u")
                    g2_sb = tmp.tile([P, B], F32, tag="g2")
                    if hc % 2 == 0:
                        nc.scalar.activation(u_sb, ps_u, AF.Silu, bias=bias_ap(COL_U + hc))
                        nc.scalar.activation(g2_sb, ps_g2, AF.Sigmoid, bias=bias_ap(COL_G2 + hc))
                    else:
                        nc.scalar.activation(g2_sb, ps_g2, AF.Sigmoid, bias=bias_ap(COL_G2 + hc))
                        nc.scalar.activation(u_sb, ps_u, AF.Silu, bias=bias_ap(COL_U + hc))
                    guf = tmp.tile([P, B], F32, tag="guf")
                    nc.vector.tensor_mul(guf, g2_sb, u_sb)
                    nc.vector.tensor_scalar_mul(gu_sb[:, hc], guf, 1.0 / GU_SCALE)
            es_gh.close()

            # ---- phase 3: y = proj_out(gu)*16 + b ----
            with tc.tile_pool(name="outp", bufs=2) as outp:
                for jc in range(JC_D):
                    ps_y = psum.tile([P, B], F32, tag="ps")
                    for half in range(2):
                        if jc == 0 and half == 0:
                            wo2_sb = wo2_first
                        else:
                            wo2_sb = wpool2.tile([P, JC_H // 2, P], F16, tag="w2m")
                            nc.sync.dma_start(out=wo2_sb[:], in_=wo2[jc, half])
                        for kk in range(JC_H // 2):
                            hc = half * (JC_H // 2) + kk
                            nc.tensor.matmul(
                                ps_y,
                                wo2_sb[:, kk],
                                gu_sb[:, hc],
                                start=(hc == 0),
                                stop=(hc == JC_H - 1),
                            )
                    y_sb = outp.tile([P, B], F32, tag="y")
                    nc.vector.tensor_scalar(
                        y_sb, ps_y, GU_SCALE, bias_ap(COL_Y + jc),
                        mybir.AluOpType.mult, mybir.AluOpType.add,
                    )
                    nc.sync.dma_start(out=out[jc], in_=y_sb[:])
            es_w2.close()
            es_gu.close()

    _split_excess_waits(nc)
    return nc


def prep_in_maps(inputs):
    """Build the 8 per-core input maps from the full-size inputs."""
    import ml_dtypes

    x = np.asarray(inputs["x"], np.float32)

    t_wo = _ternary(inputs["out_proj_w"])
    t_wu = _ternary(inputs["proj_u_w"])
    t_wg2 = _ternary(inputs["proj_g_w"])
    # fold out_proj into proj_u / proj_g: integer entries, exact in fp16
    m_u = t_wu @ t_wo  # [HID, DIM]
    m_g = t_wg2 @ t_wo
    b_o = np.asarray(inputs["out_proj_b"], np.float32)
    b_u_eff = t_wu @ b_o + np.asarray(inputs["proj_u_b"], np.float32)
    b_g_eff = t_wg2 @ b_o + np.asarray(inputs["proj_g_b"], np.float32)

    wo2_packed = _pack_weight(inputs["proj_out_w"])  # [JC_D, P, JC_H, P]
    wo2_packed = np.ascontiguousarray(
        wo2_packed.reshape(JC_D, P, 2, JC_H // 2, P).transpose(0, 2, 1, 3, 4)
    )  # [JC_D, 2, P, JC_H//2, P]

    E4 = ml_dtypes.float8_e4m3
    shared = {
        "wf": _pack_weight(inputs["f_gate_w"]),
        "wc": _pack_weight(inputs["c_proj_w"]),
        "wg": _pack_weight(inputs["g_gate_w"]),
        "wg8": _pack_weight(inputs["g_gate_w"], dtype=E4, scale=W8_SCALE),
        "mu": _pack_dense(m_u),
        "mg": _pack_dense(m_g),
        "wo2": wo2_packed,
    }
    bias = np.zeros((P, N_BIAS_COLS), np.float32)
    bias[:, COL_NF:COL_NF + JC_D] = _pack_bias_col(-np.asarray(inputs["f_gate_b"]))
    bias[:, COL_C:COL_C + JC_D] = _pack_bias_col(inputs["c_proj_b"])
    bias[:, COL_G:COL_G + JC_D] = _pack_bias_col(inputs["g_gate_b"])
    bias[:, COL_U:COL_U + JC_H] = _pack_bias_col(b_u_eff)
    bias[:, COL_G2:COL_G2 + JC_H] = _pack_bias_col(b_g_eff)
    bias[:, COL_Y:COL_Y + JC_D] = _pack_bias_col(inputs["proj_out_b"])
    shared["biases"] = bias

    in_maps = []
    for core in range(NCORES):
        m = dict(shared)
        m["xT"], m["x8"] = _pack_x(x[core * B : (core + 1) * B])
        in_maps.append(m)
    return in_maps


def gather_output(results):
    """results[i]['out'] is [JC_D, P, B]; assemble full [BATCH, DIM] f32."""
    parts = []
    for core in range(NCORES):
        y = np.asarray(results[core]["out"], np.float32)  # [jc, p, b]
        parts.append(y.reshape(DIM, B).T)  # [b, j]
    return np.ascontiguousarray(np.concatenate(parts, axis=0))


_NC_CACHE = []


def run(inputs, trace=False, **kw):
    from concourse.bass_utils import run_bass_kernel_spmd

    if not _NC_CACHE:
        _NC_CACHE.append(_build_nc())
    nc = _NC_CACHE[0]
    in_maps = prep_in_maps(inputs)
    res = run_bass_kernel_spmd(nc, in_maps, core_ids=list(range(NCORES)), trace=trace, **kw)
    return res


def kernel(**inputs):
    res = run(inputs, trace=False)
    return gather_output(res.results)
